# revision 62
# baseline (speedup 1.0000x reference)
"""Trainium2 Bass kernel for a GPT-style transformer block (B=2, T=2048, C=1024,
16 heads with the source model's direct [B,T,C]->[B,nh,T,hd] reshape).

Sharding: 8 cores; core i handles batch b=i//4 and heads [4j, 4j+4) where j=i%4.
With the direct reshape, head h's attention only reads rows [128h, 128(h+1)) of
its batch, so QKV+attention are fully core-local. Head outputs scatter over all
2048 rows; normalized bf16 y tiles are exchanged with ONE 8-way AllToAll (each
u-chunk duplicated to both batch halves; the receiving core's per-core `wo`
tensor holds zeros for cross-batch source slots, so the doubled Wo contraction
discards them), after which each core runs Wo + residual + MLP on its own 512
rows.

Attention pseudo-time runs in permuted order u = g*128 + r (model t2 = 16r + g)
so every tensor-engine operand is a direct AP slice (no transposes); the
permutation is undone on the host during output assembly.

Schedule notes: the S->exp->mask->Y chain is software-pipelined (S of gp+1 is
issued before Y of gp) so the in-order tensor queue never waits on the scalar
engine's exp; the softmax-denominator reciprocal chain of head-pair lh is
deferred into the next head's pipeline (DMA+reciprocal at gp==1, broadcast
matmuls + bf16 pack DMAs at gp==10).

Precision: bf16 operands for all GEMMs (fp32 PSUM accumulation), fp32 residual
path; rel err vs the fp32 reference ~2.4e-3 (gate 2e-2).
"""
import sys

sys.path.insert(0, "/opt/trn_rl_repo")

import numpy as np
import ml_dtypes

import concourse.bass as bass
import concourse.bacc as bacc
from concourse import tile, mybir
from concourse.bass_utils import run_bass_kernel_spmd

F32 = mybir.dt.float32
F32R = mybir.dt.float32r
BF16 = mybir.dt.bfloat16
AF = mybir.ActivationFunctionType

B, T, C = 2, 2048, 1024
GROUPS = [[0, 1, 2, 3], [4, 5, 6, 7]]
DEBUG = False
PHASES = 4  # 1=qkv, 2=+attention, 3=+pack+a2a, 4=full (timing bisection)
NO_COLLECTIVE = False  # drop the collective (timing bisection)
EXPW = 1024  # exp eviction width: 1024 (2 ACT insts/gp) or 512 (4/gp)
ATT_LEVEL = 5  # attention bisect: 1=S, 2=+exp, 3=+mask, 4=+Y, 5=+normalize
MLP_LEVEL = 3  # phase-4 bisect: 1=yt+Wo+res1, 2=+fc/gelu, 3=full


def round_fp32r(x):
    """Round fp32 -> fp32r (11-bit mantissa, RNE), keeping np.float32 storage."""
    u = np.ascontiguousarray(x, dtype=np.float32).view(np.uint32).copy()
    low = u & np.uint32(0xFFF)
    base = u & ~np.uint32(0xFFF)
    odd = ((base >> np.uint32(12)) & np.uint32(1)).astype(bool)
    up = (low > 0x800) | ((low == 0x800) & odd)
    base = base + (up.astype(np.uint32) << np.uint32(12))
    return base.view(np.float32)


def _u_rows(j):
    """Real row index t2 for each permuted column uu of core (b, j)."""
    uu = np.arange(512)
    return 16 * (uu % 128) + 4 * j + uu // 128


def _emit_body(nc, tc, P, out_p, consts, it):
    sfx = f"_{it}"
    biases, masks, ones_r, ones_b, bv, wo_sb, a2a_in, a2a_out = consts

    # ---- persistent activations (freed after the Wo phase) ----
    pers_cm = tc.tile_pool(name="persist" + sfx, bufs=1)
    pers = pers_cm.__enter__()
    qk_sb = [pers.tile([128, 512], BF16, tag="qk", bufs=16, name=f"qk{k_}{sfx}")
             for k_ in range(16)]
    qfull = pers.tile([64, 8192], BF16, tag="qfull", bufs=1, name=f"qfull{sfx}")
    kfull = pers.tile([64, 8192], BF16, tag="kfull", bufs=1, name=f"kfull{sfx}")
    v_bf = [pers.tile([128, 16, 65], BF16, tag="vbf", bufs=4, name=f"vbf{k_}{sfx}")
            for k_ in range(4)]
    y_n = [
        [pers.tile([64, 1024], BF16, tag="yn", bufs=8, name=f"yn{l_}_{k_}{sfx}")
         for k_ in range(2)]
        for l_ in range(4)
    ]

    # =============== Phase 1: QKV ===============
    with (
        tc.tile_pool(name="xtp" + sfx, bufs=1) as xtp,
        tc.tile_pool(name="wqkp" + sfx, bufs=1) as wqkp,
        tc.tile_pool(name="wvp" + sfx, bufs=1) as wvp,
        tc.tile_pool(name="qkvps" + sfx, bufs=2, space="PSUM") as qkvps,
    ):
        xt = [xtp.tile([128, 512], BF16, tag="xt", bufs=8, name=f"xt{k_}{sfx}")
              for k_ in range(8)]
        for k in range(8):
            nc.sync.dma_start(xt[k][:], P["xt"][k])

        # qk^T m-tiles (feature-major), evicted to bf16 with bias
        for half in range(2):
            wq = [wqkp.tile([128, 1024], BF16, tag="wqk", bufs=8,
                            name=f"wq{half}_{k_}{sfx}") for k_ in range(8)]
            for k in range(8):
                nc.sync.dma_start(wq[k][:], P["wqk"][half, k])
            for mi in range(8):
                m = half * 8 + mi
                ps = qkvps.tile([128, 512], F32, tag="qkv", bufs=2)
                for k in range(8):
                    nc.tensor.matmul(
                        ps[:], wq[k][:, mi * 128:(mi + 1) * 128], xt[k][:],
                        start=(k == 0), stop=(k == 7),
                    )
                nc.scalar.activation(
                    qk_sb[m][:], ps[:], AF.Identity, bias=biases[:, m:m + 1]
                )
                dst = qfull if m < 8 else kfull
                t = m if m < 8 else m - 8
                for hf in range(2):
                    g = 2 * t + hf
                    nc.sync.dma_start(
                        dst[:].rearrange("p (h x) -> p h x", h=4)[
                            :, :, g * 128:(g + 1) * 128],
                        qk_sb[m][64 * hf:64 * hf + 64, :].rearrange(
                            "p (h x) -> p h x", h=4),
                    )

        # V in row-major layout, strided into v_bf with a ones column
        wv = [wvp.tile([128, 1024], BF16, tag="wv", bufs=8, name=f"wv{k_}{sfx}")
              for k_ in range(8)]
        for k in range(8):
            nc.sync.dma_start(wv[k][:], P["wv"][k])
        for rt in range(4):
            nc.any.memset(v_bf[rt][:, :, 64:65], 1.0)
            for half in range(2):
                ps = qkvps.tile([128, 512], F32, tag="qkv", bufs=2)
                nc.tensor.matmul(
                    ps[:], ones_b[0:1, 0:128],
                    bv[0:1, half * 512:(half + 1) * 512],
                    start=True, stop=False,
                )
                for k in range(8):
                    nc.tensor.matmul(
                        ps[:], xt[k][:, rt * 128:(rt + 1) * 128],
                        wv[k][:, half * 512:(half + 1) * 512],
                        start=False, stop=(k == 7),
                    )
                nc.scalar.copy(
                    v_bf[rt][:, half * 8:(half + 1) * 8, 0:64],
                    ps[:].rearrange("p (a b) -> p a b", a=8),
                )

    if PHASES < 2:
        pers_cm.__exit__(None, None, None)
        return

    # =============== Phase 2: attention ===============
    with (
        tc.tile_pool(name="sps" + sfx, bufs=1, space="PSUM") as sps,
        tc.tile_pool(name="yps" + sfx, bufs=4, space="PSUM") as yps,
        tc.tile_pool(name="pav" + sfx, bufs=3) as pavp,
        tc.tile_pool(name="nrm" + sfx, bufs=2) as nrmp,
    ):
        def emit_s(lh, gp):
            """S matmuls + exp eviction; EXPW controls exp granularity."""
            ksl = kfull[:, lh * 2048 + gp * 128:lh * 2048 + (gp + 1) * 128]
            p_t = (pavp.tile([128, 2048], BF16, tag="p", bufs=3, name="p_t")
                   if ATT_LEVEL >= 2 else None)
            if EXPW == 1024:
                for half in range(2):
                    sp = sps.tile([128, 1024], F32, tag="s", bufs=2, name="sp")
                    for uc2 in range(2):
                        uc = half * 2 + uc2
                        qsl = qfull[:, lh * 2048 + uc * 512:
                                    lh * 2048 + (uc + 1) * 512]
                        nc.tensor.matmul(
                            sp[:, uc2 * 512:(uc2 + 1) * 512], ksl, qsl,
                            start=True, stop=True,
                        )
                    if ATT_LEVEL >= 2:
                        nc.scalar.activation(
                            p_t[:, half * 1024:(half + 1) * 1024], sp[:],
                            AF.Exp, scale=0.125,
                        )
            else:
                for uc in range(4):
                    sp = sps.tile([128, 512], F32, tag="s", bufs=4, name="sp")
                    qsl = qfull[:, lh * 2048 + uc * 512:lh * 2048 + (uc + 1) * 512]
                    nc.tensor.matmul(sp[:], ksl, qsl, start=True, stop=True)
                    if ATT_LEVEL >= 2:
                        nc.scalar.activation(
                            p_t[:, uc * 512:(uc + 1) * 512], sp[:],
                            AF.Exp, scale=0.125,
                        )
            return p_t

        def emit_mask(gp, p_t):
            for uc in range(4):
                k = min(max(gp - 4 * uc, 0), 4)
                nc.vector.tensor_mul(
                    p_t[:, uc * 512:(uc + 1) * 512],
                    p_t[:, uc * 512:(uc + 1) * 512],
                    masks[k][:],
                )

        def emit_norm_recip(yfull):
            """Start the denominator chain: DMA row 64 out, reciprocal."""
            l_sb = nrmp.tile([1, 2048], F32, tag="lsb", bufs=2, name="lsb")
            nc.sync.dma_start(l_sb[:], yfull[64:65, :])
            linv = nrmp.tile([1, 2048], F32, tag="linv", bufs=2, name="linv")
            nc.vector.reciprocal_approx_fast(linv[:], l_sb[:])
            linv_r = nrmp.tile([1, 2048], F32R, tag="linvr", bufs=2, name="linvr")
            nc.scalar.copy(linv_r[:], linv[:])
            return linv_r

        def emit_norm_apply(lh, yfull, linv_r):
            """Broadcast 1/l, scale y into the bf16 A2A tiles, pack them."""
            for uc in range(4):
                bc = sps.tile([64, 512], F32, tag="s",
                              bufs=(2 if EXPW == 1024 else 4), name="bc")
                nc.tensor.matmul(
                    bc[:], ones_r[0:1, 0:64],
                    linv_r[:, uc * 512:(uc + 1) * 512], start=True, stop=True,
                )
                h, uq = uc // 2, uc % 2
                nc.vector.tensor_mul(
                    y_n[lh][h][:, uq * 512:(uq + 1) * 512],
                    yfull[0:64, uc * 512:(uc + 1) * 512], bc[:]
                )
                if PHASES >= 3:
                    for hb in range(2):
                        nc.sync.dma_start(
                            a2a_in[4 * hb + uc, 64 * lh:64 * lh + 64, :],
                            y_n[lh][h][:, uq * 512:(uq + 1) * 512],
                        )

        norm_pending = None  # (lh, yfull, linv_r) deferred into the next lh
        for lh in range(4):
            y = ([yps.tile([65, 512], F32, tag="y", bufs=4, name=f"y{lh}_{k_}{sfx}")
                  for k_ in range(4)] if ATT_LEVEL >= 4 else None)
            # software pipeline: S(gp+1) is issued before Y(gp) so the tensor
            # engine never waits on the exp+mask chain of the current gp.
            p_cur = emit_s(lh, 0)
            if ATT_LEVEL >= 3:
                emit_mask(0, p_cur)
            for gp in range(16):
                p_nxt = emit_s(lh, gp + 1) if gp < 15 else None
                if gp == 1 and norm_pending is not None:
                    norm_pending = (*norm_pending[:2],
                                    emit_norm_recip(norm_pending[1]))
                if gp == 10 and norm_pending is not None:
                    emit_norm_apply(*norm_pending)
                    norm_pending = None
                if ATT_LEVEL >= 4:
                    for uc in range(4):
                        nc.tensor.matmul(
                            y[uc][0:65, :],
                            v_bf[lh][:, gp, :],
                            p_cur[:, uc * 512:(uc + 1) * 512],
                            start=(gp == 0), stop=(gp == 15),
                        )
                if p_nxt is not None:
                    if ATT_LEVEL >= 3:
                        emit_mask(gp + 1, p_nxt)
                    p_cur = p_nxt
            if ATT_LEVEL >= 5:
                # evict the 4 y banks now (frees them for the next lh); defer
                # the denominator chain into the next lh's pipeline.
                yfull = nrmp.tile([65, 2048], F32, tag="yev", bufs=2, name="yfull")
                for uc in range(4):
                    nc.vector.tensor_copy(
                        yfull[:, uc * 512:(uc + 1) * 512], y[uc][0:65, :]
                    )
                norm_pending = (lh, yfull)
        if norm_pending is not None:
            lh_, yfull_ = norm_pending
            emit_norm_apply(lh_, yfull_, emit_norm_recip(yfull_))

    if PHASES < 3:
        pers_cm.__exit__(None, None, None)
        return

    # =============== Phase 3: 8-way AllToAll ===============
    # a2a_in [8 dest slots, 256 feat, 512 u] was packed inside the attention
    # phase (per-lh, as soon as each head pair was normalized): chunk uc is
    # duplicated to dest slots uc and 4+uc (one lands on the same-batch core
    # that needs it; the copy received from cross-batch cores is discarded
    # downstream by zero blocks in this core's wo tensor). Features: 64*lh+d.
    pers_cm.__exit__(None, None, None)

    if not NO_COLLECTIVE:
        nc.gpsimd.collective_compute(
            "AllToAll",
            mybir.AluOpType.bypass,
            replica_groups=[[0, 1, 2, 3, 4, 5, 6, 7]],
            ins=[a2a_in.opt()],
            outs=[a2a_out.opt()],
        )

    if PHASES < 4:
        return

    # =============== Phase 4: Wo on received y, residual, MLP ===============
    with (
        tc.tile_pool(name="resp" + sfx, bufs=1) as resp,
        tc.tile_pool(name="mlp" + sfx, bufs=1) as mlpp,
    ):
        res1 = [resp.tile([128, 512], BF16, tag="res1", bufs=8,
                          name=f"res1_{k_}{sfx}") for k_ in range(8)]
        xres = [resp.tile([128, 512], F32, tag="xres", bufs=8,
                          name=f"xres{k_}{sfx}") for k_ in range(8)]
        for m in range(8):
            nc.sync.dma_start(xres[m][:], P["xres"][m])
        # y^T tiles: a2a_out[s] = 256 features of source core s for this
        # core's 512 u's. wo_sb[k] holds the matching Wo rows for same-batch
        # sources and zeros for cross-batch sources, so contracting over all
        # 16 tiles yields exactly this core's batch.
        yt = [resp.tile([128, 512], BF16, tag="yt", bufs=16,
                        name=f"yt{k_}{sfx}") for k_ in range(16)]
        for k in range(16):
            nc.sync.dma_start(
                yt[k][:], a2a_out[k // 2, (k % 2) * 128:(k % 2) * 128 + 128, :]
            )
        with tc.tile_pool(name="wops" + sfx, bufs=2, space="PSUM") as wops:
            for m in range(8):
                ps = wops.tile([128, 512], F32, tag="wo", bufs=2)
                for k in range(16):
                    nc.tensor.matmul(
                        ps[:], wo_sb[k][:, m * 128:(m + 1) * 128], yt[k][:],
                        start=(k == 0), stop=(k == 15),
                    )
                tmp = resp.tile([128, 512], F32, tag="rtmp", bufs=2)
                nc.vector.tensor_add(tmp[:], ps[:], xres[m][:])
                nc.scalar.activation(
                    res1[m][:], tmp[:], AF.Identity, bias=biases[:, 16 + m:17 + m]
                )

        if MLP_LEVEL < 2:
            return

        h1 = [mlpp.tile([128, 512], BF16, tag="h1", bufs=32, name=f"h1_{k_}{sfx}")
              for k_ in range(32)]
        h1ps_cm = tc.tile_pool(name="h1ps" + sfx, bufs=2, space="PSUM")
        mlpps = h1ps_cm.__enter__()
        for q in range(4):
            wf = [mlpp.tile([128, 1024], BF16, tag="wfc", bufs=16,
                            name=f"wf{q}_{k_}{sfx}") for k_ in range(8)]
            for k in range(8):
                nc.sync.dma_start(wf[k][:], P["wfc"][q, k])
            for mi in range(8):
                mt = q * 8 + mi
                ps = mlpps.tile([128, 512], F32, tag="h1ps", bufs=2)
                for k in range(8):
                    nc.tensor.matmul(
                        ps[:], wf[k][:, mi * 128:(mi + 1) * 128], res1[k][:],
                        start=(k == 0), stop=(k == 7),
                    )
                nc.scalar.activation(
                    h1[mt][:], ps[:], AF.Gelu_apprx_tanh,
                    bias=biases[:, 24 + mt:25 + mt],
                )
        h1ps_cm.__exit__(None, None, None)

        if MLP_LEVEL < 3:
            return

        projps_cm = tc.tile_pool(name="projps" + sfx, bufs=8, space="PSUM")
        projps = projps_cm.__enter__()
        pps = [projps.tile([128, 512], F32, tag="proj", bufs=8,
                           name=f"pps{k_}{sfx}") for k_ in range(8)]
        for k in range(32):
            wp = mlpp.tile([128, 1024], BF16, tag="wproj", bufs=3)
            nc.sync.dma_start(wp[:], P["wproj"][k])
            for m in range(8):
                nc.tensor.matmul(
                    pps[m][:], wp[:, m * 128:(m + 1) * 128], h1[k][:],
                    start=(k == 0), stop=(k == 31),
                )
        for m in range(8):
            tmp = mlpp.tile([128, 512], F32, tag="otmp", bufs=2)
            nc.vector.tensor_add(tmp[:], pps[m][:], res1[m][:])
            ob = mlpp.tile([128, 512], F32, tag="osb", bufs=2)
            nc.scalar.activation(
                ob[:], tmp[:], AF.Identity, bias=biases[:, 56 + m:57 + m]
            )
            nc.sync.dma_start(out_p[m], ob[:])
        projps_cm.__exit__(None, None, None)


def _build(iters=1):
    nc = bacc.Bacc(None, target_bir_lowering=False, debug=True, num_devices=8)

    P = {}
    P["xt"] = nc.declare_dram_parameter("xt", [8, 128, 512], BF16, isOutput=False)
    P["xres"] = nc.declare_dram_parameter("xres", [8, 128, 512], F32, isOutput=False)
    P["wqk"] = nc.declare_dram_parameter("wqk", [2, 8, 128, 1024], BF16, isOutput=False)
    P["wv"] = nc.declare_dram_parameter("wv", [8, 128, 1024], BF16, isOutput=False)
    P["bv"] = nc.declare_dram_parameter("bv", [1, 1024], BF16, isOutput=False)
    P["wo"] = nc.declare_dram_parameter("wo", [16, 128, 1024], BF16, isOutput=False)
    P["wfc"] = nc.declare_dram_parameter("wfc", [4, 8, 128, 1024], BF16, isOutput=False)
    P["wproj"] = nc.declare_dram_parameter("wproj", [32, 128, 1024], BF16, isOutput=False)
    P["biases"] = nc.declare_dram_parameter("biases", [128, 64], F32, isOutput=False)
    P["masks"] = nc.declare_dram_parameter("masks", [5, 128, 512], BF16, isOutput=False)
    out_p = nc.declare_dram_parameter("out", [8, 128, 512], F32, isOutput=True)

    with tile.TileContext(nc) as tc:
        with (
            tc.tile_pool(name="const", bufs=1) as constp,
            tc.tile_pool(name="dram", bufs=1, space="DRAM") as dram,
        ):
            biases = constp.tile([128, 64], F32, tag="biases", bufs=1)
            nc.sync.dma_start(biases[:], P["biases"][:])
            masks = [constp.tile([128, 512], BF16, tag="masks", bufs=5,
                                 name=f"masks{k_}") for k_ in range(5)]
            for k in range(5):
                nc.sync.dma_start(masks[k][:], P["masks"][k])
            ones_f = constp.tile([1, 128], F32, tag="ones_f", bufs=1)
            nc.any.memset(ones_f[:], 1.0)
            ones_r = constp.tile([1, 128], F32R, tag="ones_r", bufs=1)
            nc.scalar.copy(ones_r[:], ones_f[:])
            ones_b = constp.tile([1, 128], BF16, tag="ones_b", bufs=1)
            nc.scalar.copy(ones_b[:], ones_f[:])
            bv = constp.tile([1, 1024], BF16, tag="bv", bufs=1)
            nc.sync.dma_start(bv[:], P["bv"][:])
            wo_sb = [constp.tile([128, 1024], BF16, tag="wo", bufs=16,
                                 name=f"wo{k_}") for k_ in range(16)]
            for p_ in range(16):
                nc.sync.dma_start(wo_sb[p_][:], P["wo"][p_])

            a2a_in = dram.tile([8, 256, 512], BF16, tag="a2a_in", bufs=1)
            a2a_out = dram.tile([8, 256, 512], BF16, tag="a2a_out", bufs=1)

            consts = (biases, masks, ones_r, ones_b, bv, wo_sb, a2a_in, a2a_out)
            for it in range(iters):
                _emit_body(nc, tc, P, out_p, consts, it)

    nc.finalize()
    return nc


_NC = None


def _get_nc():
    global _NC
    if _NC is None:
        _NC = _build()
    return _NC


def _prep_inputs(x, Wqkv, bqkv, Wo, bo, Wfc, bfc, Wproj, bproj):
    x = np.asarray(x, dtype=np.float32)
    Wqkv = np.asarray(Wqkv, dtype=np.float32)
    bqkv = np.asarray(bqkv, dtype=np.float32)
    Wo_ = np.asarray(Wo, dtype=np.float32)
    Wfc = np.asarray(Wfc, dtype=np.float32)
    bfc = np.asarray(bfc, dtype=np.float32)
    Wproj = np.asarray(Wproj, dtype=np.float32)

    wqk = Wqkv[:, :2048].reshape(8, 128, 2, 1024).transpose(2, 0, 1, 3)
    wqk = np.ascontiguousarray(wqk.astype(ml_dtypes.bfloat16))
    wv = np.ascontiguousarray(
        Wqkv[:, 2048:].reshape(8, 128, 1024).astype(ml_dtypes.bfloat16))
    bv = bqkv[2048:].reshape(1, 1024).astype(ml_dtypes.bfloat16)
    wfc = Wfc.reshape(8, 128, 4, 1024).transpose(2, 0, 1, 3)
    wfc = np.ascontiguousarray(wfc.astype(ml_dtypes.bfloat16))
    wproj = np.ascontiguousarray(
        Wproj.reshape(32, 128, 1024).astype(ml_dtypes.bfloat16))

    r_ = np.arange(128)
    strict = (r_[:, None] > r_[None, :]).astype(np.float32)
    incl = (r_[:, None] >= r_[None, :]).astype(np.float32)
    masks = np.zeros((5, 128, 512), np.float32)
    for k in range(5):
        for c in range(4):
            masks[k][:, c * 128:(c + 1) * 128] = (strict if c < k else incl).T
    masks = masks.astype(ml_dtypes.bfloat16)

    wo_tiles = np.ascontiguousarray(
        Wo_.reshape(8, 128, 1024).astype(ml_dtypes.bfloat16))
    # Per-batch wo tensor [16,128,1024]: tile k matches A2A source slot
    # s=k//2 (features 256*(s%4)+128*(k%2)); zero for cross-batch slots.
    wo16 = []
    for b in range(2):
        w = np.zeros((16, 128, 1024), dtype=ml_dtypes.bfloat16)
        for k in range(16):
            s = k // 2
            if s // 4 == b:
                w[k] = wo_tiles[2 * (s % 4) + (k % 2)]
        wo16.append(w)

    biases = np.zeros((128, 64), np.float32)
    biases[:, 0:16] = bqkv[:2048].reshape(16, 128).T
    biases[:, 16:24] = np.asarray(bo, dtype=np.float32).reshape(8, 128).T
    biases[:, 24:56] = bfc.reshape(32, 128).T
    biases[:, 56:64] = np.asarray(bproj, dtype=np.float32).reshape(8, 128).T

    in_maps = []
    for i in range(8):
        j, b = i % 4, i // 4
        xq = x[b, 512 * j:512 * (j + 1), :].T.astype(ml_dtypes.bfloat16)
        xr = np.ascontiguousarray(x[b, _u_rows(j), :].T)
        in_maps.append({
            "xt": np.ascontiguousarray(xq.reshape(8, 128, 512)),
            "xres": np.ascontiguousarray(xr.reshape(8, 128, 512)),
            "wqk": wqk, "wv": wv, "bv": bv,
            "wo": wo16[b], "wfc": wfc, "wproj": wproj,
            "biases": biases, "masks": masks,
        })
    return in_maps


def _assemble(results, dtype):
    out = np.empty((B, T, C), dtype=np.float32)
    for i in range(8):
        j, b = i % 4, i // 4
        o = results[i]["out"].reshape(1024, 512)
        out[b, _u_rows(j), :] = o.T
    return out.astype(dtype, copy=False)


def kernel(**inputs):
    nc = _get_nc()
    in_maps = _prep_inputs(**inputs)
    res = run_bass_kernel_spmd(nc, in_maps, core_ids=list(range(8)))
    return _assemble(res.results, np.asarray(inputs["x"]).dtype)


if __name__ == "__main__":
    _get_nc()
    print("build ok")



# revision 67
# speedup vs baseline: 1.0889x; 1.0889x over previous
"""Trainium2 Bass kernel for a GPT-style transformer block (B=2, T=2048, C=1024,
16 heads with the source model's direct [B,T,C]->[B,nh,T,hd] reshape).

Sharding: 8 cores; core i handles batch b=i//4 and heads [4j, 4j+4) where j=i%4.
With the direct reshape, head h's attention only reads rows [128h, 128(h+1)) of
its batch, so QKV+attention are fully core-local. Head outputs scatter over all
2048 rows; normalized bf16 y tiles are exchanged with ONE 8-way AllToAll (each
u-chunk duplicated to both batch halves; the receiving core's per-core `wo`
tensor holds zeros for cross-batch source slots, so the doubled Wo contraction
discards them), after which each core runs Wo + residual + MLP on its own 512
rows.

Attention pseudo-time runs in permuted order u = g*128 + r (model t2 = 16r + g)
so every tensor-engine operand is a direct AP slice (no transposes); the
permutation is undone on the host during output assembly.

Schedule notes: the S->exp->mask->Y chain is software-pipelined (S of gp+1 is
issued before Y of gp) so the in-order tensor queue never waits on the scalar
engine's exp; the softmax-denominator reciprocal chain of head-pair lh is
deferred into the next head's pipeline (DMA+reciprocal at gp==1, broadcast
matmuls + bf16 pack DMAs at gp==10).

Precision: bf16 operands for all GEMMs (fp32 PSUM accumulation), fp32 residual
path; rel err vs the fp32 reference ~2.4e-3 (gate 2e-2).
"""
import sys

sys.path.insert(0, "/opt/trn_rl_repo")

import numpy as np
import ml_dtypes

import concourse.bass as bass
import concourse.bacc as bacc
from concourse import tile, mybir
from concourse.bass_utils import run_bass_kernel_spmd

F32 = mybir.dt.float32
F32R = mybir.dt.float32r
BF16 = mybir.dt.bfloat16
AF = mybir.ActivationFunctionType

B, T, C = 2, 2048, 1024
GROUPS = [[0, 1, 2, 3], [4, 5, 6, 7]]
DEBUG = False
PHASES = 4  # 1=qkv, 2=+attention, 3=+pack+a2a, 4=full (timing bisection)
NO_COLLECTIVE = False  # drop the collective (timing bisection)
EXPW = 1024  # exp eviction width: 1024 (2 ACT insts/gp) or 512 (4/gp)
ATT_LEVEL = 5  # attention bisect: 1=S, 2=+exp, 3=+mask, 4=+Y, 5=+normalize
MLP_LEVEL = 3  # phase-4 bisect: 1=yt+Wo+res1, 2=+fc/gelu, 3=full


def round_fp32r(x):
    """Round fp32 -> fp32r (11-bit mantissa, RNE), keeping np.float32 storage."""
    u = np.ascontiguousarray(x, dtype=np.float32).view(np.uint32).copy()
    low = u & np.uint32(0xFFF)
    base = u & ~np.uint32(0xFFF)
    odd = ((base >> np.uint32(12)) & np.uint32(1)).astype(bool)
    up = (low > 0x800) | ((low == 0x800) & odd)
    base = base + (up.astype(np.uint32) << np.uint32(12))
    return base.view(np.float32)


def _u_rows(j):
    """Real row index t2 for each permuted column uu of core (b, j)."""
    uu = np.arange(512)
    return 16 * (uu % 128) + 4 * j + uu // 128


def _emit_body(nc, tc, P, out_p, consts, it):
    sfx = f"_{it}"
    biases, masks, ones_r, ones_b, bv, wo_sb, a2a_in, a2a_out = consts

    # ---- persistent activations (freed after the Wo phase) ----
    pers_cm = tc.tile_pool(name="persist" + sfx, bufs=1)
    pers = pers_cm.__enter__()
    qk_sb = [pers.tile([128, 512], BF16, tag="qk", bufs=16, name=f"qk{k_}{sfx}")
             for k_ in range(16)]
    qfull = pers.tile([64, 8192], BF16, tag="qfull", bufs=1, name=f"qfull{sfx}")
    kfull = pers.tile([64, 8192], BF16, tag="kfull", bufs=1, name=f"kfull{sfx}")
    v_bf = [pers.tile([128, 16, 65], BF16, tag="vbf", bufs=4, name=f"vbf{k_}{sfx}")
            for k_ in range(4)]
    y_n = [
        [pers.tile([64, 1024], BF16, tag="yn", bufs=8, name=f"yn{l_}_{k_}{sfx}")
         for k_ in range(2)]
        for l_ in range(4)
    ]

    # =============== Phase 1: QKV ===============
    with (
        tc.tile_pool(name="xtp" + sfx, bufs=1) as xtp,
        tc.tile_pool(name="wqkp" + sfx, bufs=1) as wqkp,
        tc.tile_pool(name="wvp" + sfx, bufs=1) as wvp,
        tc.tile_pool(name="qkvps" + sfx, bufs=2, space="PSUM") as qkvps,
    ):
        xt = [xtp.tile([128, 512], BF16, tag="xt", bufs=8, name=f"xt{k_}{sfx}")
              for k_ in range(8)]
        for k in range(8):
            nc.sync.dma_start(xt[k][:], P["xt"][k])

        # qk^T m-tiles (feature-major), evicted to bf16 with bias
        for half in range(2):
            wq = [wqkp.tile([128, 1024], BF16, tag="wqk", bufs=8,
                            name=f"wq{half}_{k_}{sfx}") for k_ in range(8)]
            for k in range(8):
                nc.sync.dma_start(wq[k][:], P["wqk"][half, k])
            for mi in range(8):
                m = half * 8 + mi
                ps = qkvps.tile([128, 512], F32, tag="qkv", bufs=2)
                for k in range(8):
                    nc.tensor.matmul(
                        ps[:], wq[k][:, mi * 128:(mi + 1) * 128], xt[k][:],
                        start=(k == 0), stop=(k == 7),
                    )
                nc.scalar.activation(
                    qk_sb[m][:], ps[:], AF.Identity, bias=biases[:, m:m + 1]
                )
                dst = qfull if m < 8 else kfull
                t = m if m < 8 else m - 8
                # ACT-ring DMAs: don't queue behind the SP ring's bulk weight
                # streams — these shuffles gate the start of attention.
                for hf in range(2):
                    g = 2 * t + hf
                    nc.scalar.dma_start(
                        dst[:].rearrange("p (h x) -> p h x", h=4)[
                            :, :, g * 128:(g + 1) * 128],
                        qk_sb[m][64 * hf:64 * hf + 64, :].rearrange(
                            "p (h x) -> p h x", h=4),
                    )

        # V in row-major layout, strided into v_bf with a ones column
        wv = [wvp.tile([128, 1024], BF16, tag="wv", bufs=8, name=f"wv{k_}{sfx}")
              for k_ in range(8)]
        for k in range(8):
            nc.sync.dma_start(wv[k][:], P["wv"][k])
        for rt in range(4):
            nc.any.memset(v_bf[rt][:, :, 64:65], 1.0)
            for half in range(2):
                ps = qkvps.tile([128, 512], F32, tag="qkv", bufs=2)
                nc.tensor.matmul(
                    ps[:], ones_b[0:1, 0:128],
                    bv[0:1, half * 512:(half + 1) * 512],
                    start=True, stop=False,
                )
                for k in range(8):
                    nc.tensor.matmul(
                        ps[:], xt[k][:, rt * 128:(rt + 1) * 128],
                        wv[k][:, half * 512:(half + 1) * 512],
                        start=False, stop=(k == 7),
                    )
                nc.scalar.copy(
                    v_bf[rt][:, half * 8:(half + 1) * 8, 0:64],
                    ps[:].rearrange("p (a b) -> p a b", a=8),
                )

    if PHASES < 2:
        pers_cm.__exit__(None, None, None)
        return

    # =============== Phase 2: attention ===============
    with (
        tc.tile_pool(name="sps" + sfx, bufs=1, space="PSUM") as sps,
        tc.tile_pool(name="yps" + sfx, bufs=4, space="PSUM") as yps,
        tc.tile_pool(name="pav" + sfx, bufs=3) as pavp,
        tc.tile_pool(name="nrm" + sfx, bufs=2) as nrmp,
    ):
        def emit_s(lh, gp):
            """S matmuls + exp eviction; EXPW controls exp granularity."""
            ksl = kfull[:, lh * 2048 + gp * 128:lh * 2048 + (gp + 1) * 128]
            p_t = (pavp.tile([128, 2048], BF16, tag="p", bufs=3, name="p_t")
                   if ATT_LEVEL >= 2 else None)
            if EXPW == 1024:
                for half in range(2):
                    sp = sps.tile([128, 1024], F32, tag="s", bufs=2, name="sp")
                    for uc2 in range(2):
                        uc = half * 2 + uc2
                        qsl = qfull[:, lh * 2048 + uc * 512:
                                    lh * 2048 + (uc + 1) * 512]
                        nc.tensor.matmul(
                            sp[:, uc2 * 512:(uc2 + 1) * 512], ksl, qsl,
                            start=True, stop=True,
                        )
                    if ATT_LEVEL >= 2:
                        nc.scalar.activation(
                            p_t[:, half * 1024:(half + 1) * 1024], sp[:],
                            AF.Exp, scale=0.125,
                        )
            else:
                for uc in range(4):
                    sp = sps.tile([128, 512], F32, tag="s", bufs=4, name="sp")
                    qsl = qfull[:, lh * 2048 + uc * 512:lh * 2048 + (uc + 1) * 512]
                    nc.tensor.matmul(sp[:], ksl, qsl, start=True, stop=True)
                    if ATT_LEVEL >= 2:
                        nc.scalar.activation(
                            p_t[:, uc * 512:(uc + 1) * 512], sp[:],
                            AF.Exp, scale=0.125,
                        )
            return p_t

        def emit_mask(gp, p_t):
            for uc in range(4):
                k = min(max(gp - 4 * uc, 0), 4)
                nc.vector.tensor_mul(
                    p_t[:, uc * 512:(uc + 1) * 512],
                    p_t[:, uc * 512:(uc + 1) * 512],
                    masks[k][:],
                )

        def emit_norm_recip(yfull):
            """Start the denominator chain: DMA row 64 out, reciprocal."""
            l_sb = nrmp.tile([1, 2048], F32, tag="lsb", bufs=2, name="lsb")
            nc.scalar.dma_start(l_sb[:], yfull[64:65, :])
            linv = nrmp.tile([1, 2048], F32, tag="linv", bufs=2, name="linv")
            nc.vector.reciprocal_approx_fast(linv[:], l_sb[:])
            linv_r = nrmp.tile([1, 2048], F32R, tag="linvr", bufs=2, name="linvr")
            nc.scalar.copy(linv_r[:], linv[:])
            return linv_r

        def emit_norm_apply(lh, yfull, linv_r):
            """Broadcast 1/l, scale y into the bf16 A2A tiles, pack them."""
            for uc in range(4):
                bc = sps.tile([64, 512], F32, tag="s",
                              bufs=(2 if EXPW == 1024 else 4), name="bc")
                nc.tensor.matmul(
                    bc[:], ones_r[0:1, 0:64],
                    linv_r[:, uc * 512:(uc + 1) * 512], start=True, stop=True,
                )
                h, uq = uc // 2, uc % 2
                nc.vector.tensor_mul(
                    y_n[lh][h][:, uq * 512:(uq + 1) * 512],
                    yfull[0:64, uc * 512:(uc + 1) * 512], bc[:]
                )
                if PHASES >= 3:
                    for hb in range(2):
                        nc.scalar.dma_start(
                            a2a_in[4 * hb + uc, 64 * lh:64 * lh + 64, :],
                            y_n[lh][h][:, uq * 512:(uq + 1) * 512],
                        )

        norm_pending = None  # (lh, yfull, linv_r) deferred into the next lh
        for lh in range(4):
            y = ([yps.tile([65, 512], F32, tag="y", bufs=4, name=f"y{lh}_{k_}{sfx}")
                  for k_ in range(4)] if ATT_LEVEL >= 4 else None)
            # software pipeline: S(gp+1) is issued before Y(gp) so the tensor
            # engine never waits on the exp+mask chain of the current gp.
            p_cur = emit_s(lh, 0)
            if ATT_LEVEL >= 3:
                emit_mask(0, p_cur)
            for gp in range(16):
                p_nxt = emit_s(lh, gp + 1) if gp < 15 else None
                if gp == 1 and norm_pending is not None:
                    norm_pending = (*norm_pending[:2],
                                    emit_norm_recip(norm_pending[1]))
                if gp == 10 and norm_pending is not None:
                    emit_norm_apply(*norm_pending)
                    norm_pending = None
                if ATT_LEVEL >= 4:
                    for uc in range(4):
                        nc.tensor.matmul(
                            y[uc][0:65, :],
                            v_bf[lh][:, gp, :],
                            p_cur[:, uc * 512:(uc + 1) * 512],
                            start=(gp == 0), stop=(gp == 15),
                        )
                if p_nxt is not None:
                    if ATT_LEVEL >= 3:
                        emit_mask(gp + 1, p_nxt)
                    p_cur = p_nxt
            if ATT_LEVEL >= 5:
                # evict the 4 y banks now (frees them for the next lh); defer
                # the denominator chain into the next lh's pipeline.
                yfull = nrmp.tile([65, 2048], F32, tag="yev", bufs=2, name="yfull")
                for uc in range(4):
                    nc.vector.tensor_copy(
                        yfull[:, uc * 512:(uc + 1) * 512], y[uc][0:65, :]
                    )
                norm_pending = (lh, yfull)
        if norm_pending is not None:
            lh_, yfull_ = norm_pending
            emit_norm_apply(lh_, yfull_, emit_norm_recip(yfull_))

    if PHASES < 3:
        pers_cm.__exit__(None, None, None)
        return

    # =============== Phase 3: 8-way AllToAll ===============
    # a2a_in [8 dest slots, 256 feat, 512 u] was packed inside the attention
    # phase (per-lh, as soon as each head pair was normalized): chunk uc is
    # duplicated to dest slots uc and 4+uc (one lands on the same-batch core
    # that needs it; the copy received from cross-batch cores is discarded
    # downstream by zero blocks in this core's wo tensor). Features: 64*lh+d.
    pers_cm.__exit__(None, None, None)

    if not NO_COLLECTIVE:
        nc.gpsimd.collective_compute(
            "AllToAll",
            mybir.AluOpType.bypass,
            replica_groups=[[0, 1, 2, 3, 4, 5, 6, 7]],
            ins=[a2a_in.opt()],
            outs=[a2a_out.opt()],
        )

    if PHASES < 4:
        return

    # =============== Phase 4: Wo on received y, residual, MLP ===============
    with (
        tc.tile_pool(name="resp" + sfx, bufs=1) as resp,
        tc.tile_pool(name="mlp" + sfx, bufs=1) as mlpp,
    ):
        res1 = [resp.tile([128, 512], BF16, tag="res1", bufs=8,
                          name=f"res1_{k_}{sfx}") for k_ in range(8)]
        xres = [resp.tile([128, 512], F32, tag="xres", bufs=8,
                          name=f"xres{k_}{sfx}") for k_ in range(8)]
        for m in range(8):
            nc.scalar.dma_start(xres[m][:], P["xres"][m])
        # y^T tiles: a2a_out[s] = 256 features of source core s for this
        # core's 512 u's. wo_sb[k] holds the matching Wo rows for same-batch
        # sources and zeros for cross-batch sources, so contracting over all
        # 16 tiles yields exactly this core's batch.
        yt = [resp.tile([128, 512], BF16, tag="yt", bufs=16,
                        name=f"yt{k_}{sfx}") for k_ in range(16)]
        for k in range(16):
            nc.scalar.dma_start(
                yt[k][:], a2a_out[k // 2, (k % 2) * 128:(k % 2) * 128 + 128, :]
            )
        with tc.tile_pool(name="wops" + sfx, bufs=2, space="PSUM") as wops:
            for m in range(8):
                ps = wops.tile([128, 512], F32, tag="wo", bufs=2)
                for k in range(16):
                    nc.tensor.matmul(
                        ps[:], wo_sb[k][:, m * 128:(m + 1) * 128], yt[k][:],
                        start=(k == 0), stop=(k == 15),
                    )
                tmp = resp.tile([128, 512], F32, tag="rtmp", bufs=2)
                nc.vector.tensor_add(tmp[:], ps[:], xres[m][:])
                nc.scalar.activation(
                    res1[m][:], tmp[:], AF.Identity, bias=biases[:, 16 + m:17 + m]
                )

        if MLP_LEVEL < 2:
            return

        h1 = [mlpp.tile([128, 512], BF16, tag="h1", bufs=32, name=f"h1_{k_}{sfx}")
              for k_ in range(32)]
        h1ps_cm = tc.tile_pool(name="h1ps" + sfx, bufs=2, space="PSUM")
        mlpps = h1ps_cm.__enter__()
        for q in range(4):
            wf = [mlpp.tile([128, 1024], BF16, tag="wfc", bufs=16,
                            name=f"wf{q}_{k_}{sfx}") for k_ in range(8)]
            for k in range(8):
                nc.sync.dma_start(wf[k][:], P["wfc"][q, k])
            for mi in range(8):
                mt = q * 8 + mi
                ps = mlpps.tile([128, 512], F32, tag="h1ps", bufs=2)
                for k in range(8):
                    nc.tensor.matmul(
                        ps[:], wf[k][:, mi * 128:(mi + 1) * 128], res1[k][:],
                        start=(k == 0), stop=(k == 7),
                    )
                nc.scalar.activation(
                    h1[mt][:], ps[:], AF.Gelu_apprx_tanh,
                    bias=biases[:, 24 + mt:25 + mt],
                )
        h1ps_cm.__exit__(None, None, None)

        if MLP_LEVEL < 3:
            return

        projps_cm = tc.tile_pool(name="projps" + sfx, bufs=8, space="PSUM")
        projps = projps_cm.__enter__()
        pps = [projps.tile([128, 512], F32, tag="proj", bufs=8,
                           name=f"pps{k_}{sfx}") for k_ in range(8)]
        for k in range(32):
            wp = mlpp.tile([128, 1024], BF16, tag="wproj", bufs=3)
            nc.sync.dma_start(wp[:], P["wproj"][k])
            for m in range(8):
                nc.tensor.matmul(
                    pps[m][:], wp[:, m * 128:(m + 1) * 128], h1[k][:],
                    start=(k == 0), stop=(k == 31),
                )
        for m in range(8):
            tmp = mlpp.tile([128, 512], F32, tag="otmp", bufs=2)
            nc.vector.tensor_add(tmp[:], pps[m][:], res1[m][:])
            ob = mlpp.tile([128, 512], F32, tag="osb", bufs=2)
            nc.scalar.activation(
                ob[:], tmp[:], AF.Identity, bias=biases[:, 56 + m:57 + m]
            )
            nc.sync.dma_start(out_p[m], ob[:])
        projps_cm.__exit__(None, None, None)


def _build(iters=1):
    nc = bacc.Bacc(None, target_bir_lowering=False, debug=True, num_devices=8)

    P = {}
    P["xt"] = nc.declare_dram_parameter("xt", [8, 128, 512], BF16, isOutput=False)
    P["xres"] = nc.declare_dram_parameter("xres", [8, 128, 512], F32, isOutput=False)
    P["wqk"] = nc.declare_dram_parameter("wqk", [2, 8, 128, 1024], BF16, isOutput=False)
    P["wv"] = nc.declare_dram_parameter("wv", [8, 128, 1024], BF16, isOutput=False)
    P["bv"] = nc.declare_dram_parameter("bv", [1, 1024], BF16, isOutput=False)
    P["wo"] = nc.declare_dram_parameter("wo", [16, 128, 1024], BF16, isOutput=False)
    P["wfc"] = nc.declare_dram_parameter("wfc", [4, 8, 128, 1024], BF16, isOutput=False)
    P["wproj"] = nc.declare_dram_parameter("wproj", [32, 128, 1024], BF16, isOutput=False)
    P["biases"] = nc.declare_dram_parameter("biases", [128, 64], F32, isOutput=False)
    P["masks"] = nc.declare_dram_parameter("masks", [5, 128, 512], BF16, isOutput=False)
    out_p = nc.declare_dram_parameter("out", [8, 128, 512], F32, isOutput=True)

    with tile.TileContext(nc) as tc:
        with (
            tc.tile_pool(name="const", bufs=1) as constp,
            tc.tile_pool(name="dram", bufs=1, space="DRAM") as dram,
        ):
            biases = constp.tile([128, 64], F32, tag="biases", bufs=1)
            nc.sync.dma_start(biases[:], P["biases"][:])
            masks = [constp.tile([128, 512], BF16, tag="masks", bufs=5,
                                 name=f"masks{k_}") for k_ in range(5)]
            for k in range(5):
                nc.sync.dma_start(masks[k][:], P["masks"][k])
            ones_f = constp.tile([1, 128], F32, tag="ones_f", bufs=1)
            nc.any.memset(ones_f[:], 1.0)
            ones_r = constp.tile([1, 128], F32R, tag="ones_r", bufs=1)
            nc.scalar.copy(ones_r[:], ones_f[:])
            ones_b = constp.tile([1, 128], BF16, tag="ones_b", bufs=1)
            nc.scalar.copy(ones_b[:], ones_f[:])
            bv = constp.tile([1, 1024], BF16, tag="bv", bufs=1)
            nc.sync.dma_start(bv[:], P["bv"][:])
            wo_sb = [constp.tile([128, 1024], BF16, tag="wo", bufs=16,
                                 name=f"wo{k_}") for k_ in range(16)]
            for p_ in range(16):
                nc.sync.dma_start(wo_sb[p_][:], P["wo"][p_])

            a2a_in = dram.tile([8, 256, 512], BF16, tag="a2a_in", bufs=1)
            a2a_out = dram.tile([8, 256, 512], BF16, tag="a2a_out", bufs=1)

            consts = (biases, masks, ones_r, ones_b, bv, wo_sb, a2a_in, a2a_out)
            for it in range(iters):
                _emit_body(nc, tc, P, out_p, consts, it)

    nc.finalize()
    return nc


_NC = None


def _get_nc():
    global _NC
    if _NC is None:
        _NC = _build()
    return _NC


def _prep_inputs(x, Wqkv, bqkv, Wo, bo, Wfc, bfc, Wproj, bproj):
    x = np.asarray(x, dtype=np.float32)
    Wqkv = np.asarray(Wqkv, dtype=np.float32)
    bqkv = np.asarray(bqkv, dtype=np.float32)
    Wo_ = np.asarray(Wo, dtype=np.float32)
    Wfc = np.asarray(Wfc, dtype=np.float32)
    bfc = np.asarray(bfc, dtype=np.float32)
    Wproj = np.asarray(Wproj, dtype=np.float32)

    wqk = Wqkv[:, :2048].reshape(8, 128, 2, 1024).transpose(2, 0, 1, 3)
    wqk = np.ascontiguousarray(wqk.astype(ml_dtypes.bfloat16))
    wv = np.ascontiguousarray(
        Wqkv[:, 2048:].reshape(8, 128, 1024).astype(ml_dtypes.bfloat16))
    bv = bqkv[2048:].reshape(1, 1024).astype(ml_dtypes.bfloat16)
    wfc = Wfc.reshape(8, 128, 4, 1024).transpose(2, 0, 1, 3)
    wfc = np.ascontiguousarray(wfc.astype(ml_dtypes.bfloat16))
    wproj = np.ascontiguousarray(
        Wproj.reshape(32, 128, 1024).astype(ml_dtypes.bfloat16))

    r_ = np.arange(128)
    strict = (r_[:, None] > r_[None, :]).astype(np.float32)
    incl = (r_[:, None] >= r_[None, :]).astype(np.float32)
    masks = np.zeros((5, 128, 512), np.float32)
    for k in range(5):
        for c in range(4):
            masks[k][:, c * 128:(c + 1) * 128] = (strict if c < k else incl).T
    masks = masks.astype(ml_dtypes.bfloat16)

    wo_tiles = np.ascontiguousarray(
        Wo_.reshape(8, 128, 1024).astype(ml_dtypes.bfloat16))
    # Per-batch wo tensor [16,128,1024]: tile k matches A2A source slot
    # s=k//2 (features 256*(s%4)+128*(k%2)); zero for cross-batch slots.
    wo16 = []
    for b in range(2):
        w = np.zeros((16, 128, 1024), dtype=ml_dtypes.bfloat16)
        for k in range(16):
            s = k // 2
            if s // 4 == b:
                w[k] = wo_tiles[2 * (s % 4) + (k % 2)]
        wo16.append(w)

    biases = np.zeros((128, 64), np.float32)
    biases[:, 0:16] = bqkv[:2048].reshape(16, 128).T
    biases[:, 16:24] = np.asarray(bo, dtype=np.float32).reshape(8, 128).T
    biases[:, 24:56] = bfc.reshape(32, 128).T
    biases[:, 56:64] = np.asarray(bproj, dtype=np.float32).reshape(8, 128).T

    in_maps = []
    for i in range(8):
        j, b = i % 4, i // 4
        xq = x[b, 512 * j:512 * (j + 1), :].T.astype(ml_dtypes.bfloat16)
        xr = np.ascontiguousarray(x[b, _u_rows(j), :].T)
        in_maps.append({
            "xt": np.ascontiguousarray(xq.reshape(8, 128, 512)),
            "xres": np.ascontiguousarray(xr.reshape(8, 128, 512)),
            "wqk": wqk, "wv": wv, "bv": bv,
            "wo": wo16[b], "wfc": wfc, "wproj": wproj,
            "biases": biases, "masks": masks,
        })
    return in_maps


def _assemble(results, dtype):
    out = np.empty((B, T, C), dtype=np.float32)
    for i in range(8):
        j, b = i % 4, i // 4
        o = results[i]["out"].reshape(1024, 512)
        out[b, _u_rows(j), :] = o.T
    return out.astype(dtype, copy=False)


def kernel(**inputs):
    nc = _get_nc()
    in_maps = _prep_inputs(**inputs)
    res = run_bass_kernel_spmd(nc, in_maps, core_ids=list(range(8)))
    return _assemble(res.results, np.asarray(inputs["x"]).dtype)


if __name__ == "__main__":
    _get_nc()
    print("build ok")



# revision 69
# speedup vs baseline: 1.2655x; 1.1622x over previous
"""Trainium2 Bass kernel for a GPT-style transformer block (B=2, T=2048, C=1024,
16 heads with the source model's direct [B,T,C]->[B,nh,T,hd] reshape).

Sharding: 8 cores; core i handles batch b=i//4 and heads [4j, 4j+4) where j=i%4.
With the direct reshape, head h's attention only reads rows [128h, 128(h+1)) of
its batch, so QKV+attention are fully core-local. Head outputs scatter over all
2048 rows; normalized bf16 y tiles are exchanged with ONE 8-way AllToAll (each
u-chunk duplicated to both batch halves; the receiving core's per-core `wo`
tensor holds zeros for cross-batch source slots, so the doubled Wo contraction
discards them), after which each core runs Wo + residual + MLP on its own 512
rows.

Attention pseudo-time runs in permuted order u = g*128 + r (model t2 = 16r + g)
so every tensor-engine operand is a direct AP slice (no transposes); the
permutation is undone on the host during output assembly.

Schedule notes: the S->exp->mask->Y chain is software-pipelined (S of gp+1 is
issued before Y of gp) so the in-order tensor queue never waits on the scalar
engine's exp; the softmax-denominator reciprocal chain of head-pair lh is
deferred into the next head's pipeline (DMA+reciprocal at gp==1, broadcast
matmuls + bf16 pack DMAs at gp==10).

Precision: bf16 operands for all GEMMs (fp32 PSUM accumulation), fp32 residual
path; rel err vs the fp32 reference ~2.4e-3 (gate 2e-2).
"""
import sys

sys.path.insert(0, "/opt/trn_rl_repo")

import numpy as np
import ml_dtypes

import concourse.bass as bass
import concourse.bacc as bacc
from concourse import tile, mybir
from concourse.bass_utils import run_bass_kernel_spmd

F32 = mybir.dt.float32
F32R = mybir.dt.float32r
BF16 = mybir.dt.bfloat16
AF = mybir.ActivationFunctionType

B, T, C = 2, 2048, 1024
GROUPS = [[0, 1, 2, 3], [4, 5, 6, 7]]
DEBUG = False
PHASES = 4  # 1=qkv, 2=+attention, 3=+pack+a2a, 4=full (timing bisection)
NO_COLLECTIVE = False  # drop the collective (timing bisection)
EXPW = 1024  # exp eviction width: 1024 (2 ACT insts/gp) or 512 (4/gp)
ATT_LEVEL = 5  # attention bisect: 1=S, 2=+exp, 3=+mask, 4=+Y, 5=+normalize
MLP_LEVEL = 3  # phase-4 bisect: 1=yt+Wo+res1, 2=+fc/gelu, 3=full


def round_fp32r(x):
    """Round fp32 -> fp32r (11-bit mantissa, RNE), keeping np.float32 storage."""
    u = np.ascontiguousarray(x, dtype=np.float32).view(np.uint32).copy()
    low = u & np.uint32(0xFFF)
    base = u & ~np.uint32(0xFFF)
    odd = ((base >> np.uint32(12)) & np.uint32(1)).astype(bool)
    up = (low > 0x800) | ((low == 0x800) & odd)
    base = base + (up.astype(np.uint32) << np.uint32(12))
    return base.view(np.float32)


def _u_rows(j):
    """Real row index t2 for each permuted column uu of core (b, j)."""
    uu = np.arange(512)
    return 16 * (uu % 128) + 4 * j + uu // 128


def _emit_body(nc, tc, P, out_p, consts, it):
    sfx = f"_{it}"
    biases, masks, ones_r, ones_b, bv, wo_sb, a2a_in, a2a_out = consts

    # ---- persistent activations (freed after the Wo phase) ----
    pers_cm = tc.tile_pool(name="persist" + sfx, bufs=1)
    pers = pers_cm.__enter__()
    qk_sb = [pers.tile([128, 512], BF16, tag="qk", bufs=16, name=f"qk{k_}{sfx}")
             for k_ in range(16)]
    qfull = pers.tile([64, 8192], BF16, tag="qfull", bufs=1, name=f"qfull{sfx}")
    kfull = pers.tile([64, 8192], BF16, tag="kfull", bufs=1, name=f"kfull{sfx}")
    v_bf = [pers.tile([128, 16, 65], BF16, tag="vbf", bufs=4, name=f"vbf{k_}{sfx}")
            for k_ in range(4)]
    y_n = [
        [pers.tile([64, 1024], BF16, tag="yn", bufs=8, name=f"yn{l_}_{k_}{sfx}")
         for k_ in range(2)]
        for l_ in range(4)
    ]

    # =============== Phase 1: QKV ===============
    with (
        tc.tile_pool(name="xtp" + sfx, bufs=1) as xtp,
        tc.tile_pool(name="wqkp" + sfx, bufs=1) as wqkp,
        tc.tile_pool(name="wvp" + sfx, bufs=1) as wvp,
        tc.tile_pool(name="qkvps" + sfx, bufs=2, space="PSUM") as qkvps,
    ):
        xt = [xtp.tile([128, 512], BF16, tag="xt", bufs=8, name=f"xt{k_}{sfx}")
              for k_ in range(8)]
        for k in range(8):
            nc.sync.dma_start(xt[k][:], P["xt"][k])

        # qk^T m-tiles (feature-major), evicted to bf16 with bias
        for half in range(2):
            wq = [wqkp.tile([128, 1024], BF16, tag="wqk", bufs=8,
                            name=f"wq{half}_{k_}{sfx}") for k_ in range(8)]
            for k in range(8):
                nc.sync.dma_start(wq[k][:], P["wqk"][half, k])
            for mi in range(8):
                m = half * 8 + mi
                ps = qkvps.tile([128, 512], F32, tag="qkv", bufs=2)
                for k in range(8):
                    nc.tensor.matmul(
                        ps[:], wq[k][:, mi * 128:(mi + 1) * 128], xt[k][:],
                        start=(k == 0), stop=(k == 7),
                    )
                nc.scalar.activation(
                    qk_sb[m][:], ps[:], AF.Identity, bias=biases[:, m:m + 1]
                )
                dst = qfull if m < 8 else kfull
                t = m if m < 8 else m - 8
                # ACT-ring DMAs: don't queue behind the SP ring's bulk weight
                # streams — these shuffles gate the start of attention.
                for hf in range(2):
                    g = 2 * t + hf
                    nc.scalar.dma_start(
                        dst[:].rearrange("p (h x) -> p h x", h=4)[
                            :, :, g * 128:(g + 1) * 128],
                        qk_sb[m][64 * hf:64 * hf + 64, :].rearrange(
                            "p (h x) -> p h x", h=4),
                    )

        # V in row-major layout, strided into v_bf with a ones column
        wv = [wvp.tile([128, 1024], BF16, tag="wv", bufs=8, name=f"wv{k_}{sfx}")
              for k_ in range(8)]
        for k in range(8):
            nc.sync.dma_start(wv[k][:], P["wv"][k])
        for rt in range(4):
            nc.any.memset(v_bf[rt][:, :, 64:65], 1.0)
            for half in range(2):
                ps = qkvps.tile([128, 512], F32, tag="qkv", bufs=2)
                nc.tensor.matmul(
                    ps[:], ones_b[0:1, 0:128],
                    bv[0:1, half * 512:(half + 1) * 512],
                    start=True, stop=False,
                )
                for k in range(8):
                    nc.tensor.matmul(
                        ps[:], xt[k][:, rt * 128:(rt + 1) * 128],
                        wv[k][:, half * 512:(half + 1) * 512],
                        start=False, stop=(k == 7),
                    )
                nc.scalar.copy(
                    v_bf[rt][:, half * 8:(half + 1) * 8, 0:64],
                    ps[:].rearrange("p (a b) -> p a b", a=8),
                )

    if PHASES < 2:
        pers_cm.__exit__(None, None, None)
        return

    # =============== Phase 2: attention ===============
    with (
        tc.tile_pool(name="sps" + sfx, bufs=1, space="PSUM") as sps,
        tc.tile_pool(name="yps" + sfx, bufs=4, space="PSUM") as yps,
        tc.tile_pool(name="pav" + sfx, bufs=3) as pavp,
        tc.tile_pool(name="nrm" + sfx, bufs=2) as nrmp,
    ):
        def emit_s(lh, gp):
            """S matmuls + exp eviction; EXPW controls exp granularity."""
            ksl = kfull[:, lh * 2048 + gp * 128:lh * 2048 + (gp + 1) * 128]
            p_t = (pavp.tile([128, 2048], BF16, tag="p", bufs=3, name="p_t")
                   if ATT_LEVEL >= 2 else None)
            if EXPW == 1024:
                for half in range(2):
                    sp = sps.tile([128, 1024], F32, tag="s", bufs=2, name="sp")
                    for uc2 in range(2):
                        uc = half * 2 + uc2
                        qsl = qfull[:, lh * 2048 + uc * 512:
                                    lh * 2048 + (uc + 1) * 512]
                        nc.tensor.matmul(
                            sp[:, uc2 * 512:(uc2 + 1) * 512], ksl, qsl,
                            start=True, stop=True,
                        )
                    if ATT_LEVEL >= 2:
                        nc.scalar.activation(
                            p_t[:, half * 1024:(half + 1) * 1024], sp[:],
                            AF.Exp, scale=0.125,
                        )
            else:
                for uc in range(4):
                    sp = sps.tile([128, 512], F32, tag="s", bufs=4, name="sp")
                    qsl = qfull[:, lh * 2048 + uc * 512:lh * 2048 + (uc + 1) * 512]
                    nc.tensor.matmul(sp[:], ksl, qsl, start=True, stop=True)
                    if ATT_LEVEL >= 2:
                        nc.scalar.activation(
                            p_t[:, uc * 512:(uc + 1) * 512], sp[:],
                            AF.Exp, scale=0.125,
                        )
            return p_t

        def emit_mask(gp, p_t):
            for uc in range(4):
                k = min(max(gp - 4 * uc, 0), 4)
                nc.vector.tensor_mul(
                    p_t[:, uc * 512:(uc + 1) * 512],
                    p_t[:, uc * 512:(uc + 1) * 512],
                    masks[k][:],
                )

        def emit_norm_recip(yfull):
            """Start the denominator chain: DMA row 64 out, reciprocal."""
            l_sb = nrmp.tile([1, 2048], F32, tag="lsb", bufs=2, name="lsb")
            nc.scalar.dma_start(l_sb[:], yfull[64:65, :])
            linv = nrmp.tile([1, 2048], F32, tag="linv", bufs=2, name="linv")
            nc.vector.reciprocal_approx_fast(linv[:], l_sb[:])
            linv_r = nrmp.tile([1, 2048], F32R, tag="linvr", bufs=2, name="linvr")
            nc.scalar.copy(linv_r[:], linv[:])
            return linv_r

        def emit_norm_apply(lh, yfull, linv_r):
            """Broadcast 1/l, scale y into the bf16 A2A tiles, pack them."""
            for uc in range(4):
                bc = sps.tile([64, 512], F32, tag="s",
                              bufs=(2 if EXPW == 1024 else 4), name="bc")
                nc.tensor.matmul(
                    bc[:], ones_r[0:1, 0:64],
                    linv_r[:, uc * 512:(uc + 1) * 512], start=True, stop=True,
                )
                h, uq = uc // 2, uc % 2
                nc.vector.tensor_mul(
                    y_n[lh][h][:, uq * 512:(uq + 1) * 512],
                    yfull[0:64, uc * 512:(uc + 1) * 512], bc[:]
                )
                if PHASES >= 3:
                    for hb in range(2):
                        nc.scalar.dma_start(
                            a2a_in[4 * hb + uc, 64 * lh:64 * lh + 64, :],
                            y_n[lh][h][:, uq * 512:(uq + 1) * 512],
                        )

        norm_pending = None  # (lh, yfull, linv_r) deferred into the next lh
        for lh in range(4):
            y = ([yps.tile([65, 512], F32, tag="y", bufs=4, name=f"y{lh}_{k_}{sfx}")
                  for k_ in range(4)] if ATT_LEVEL >= 4 else None)
            # software pipeline: S(gp+1) is issued before Y(gp) so the tensor
            # engine never waits on the exp+mask chain of the current gp.
            p_cur = emit_s(lh, 0)
            if ATT_LEVEL >= 3:
                emit_mask(0, p_cur)
            for gp in range(16):
                p_nxt = emit_s(lh, gp + 1) if gp < 15 else None
                if gp == 1 and norm_pending is not None:
                    norm_pending = (*norm_pending[:2],
                                    emit_norm_recip(norm_pending[1]))
                if gp == 10 and norm_pending is not None:
                    emit_norm_apply(*norm_pending)
                    norm_pending = None
                if ATT_LEVEL >= 4:
                    for uc in range(4):
                        nc.tensor.matmul(
                            y[uc][0:65, :],
                            v_bf[lh][:, gp, :],
                            p_cur[:, uc * 512:(uc + 1) * 512],
                            start=(gp == 0), stop=(gp == 15),
                        )
                if p_nxt is not None:
                    if ATT_LEVEL >= 3:
                        emit_mask(gp + 1, p_nxt)
                    p_cur = p_nxt
            if ATT_LEVEL >= 5:
                # evict the 4 y banks now (frees them for the next lh); defer
                # the denominator chain into the next lh's pipeline.
                yfull = nrmp.tile([65, 2048], F32, tag="yev", bufs=2, name="yfull")
                for uc in range(4):
                    nc.vector.tensor_copy(
                        yfull[:, uc * 512:(uc + 1) * 512], y[uc][0:65, :]
                    )
                norm_pending = (lh, yfull)
        if norm_pending is not None:
            lh_, yfull_ = norm_pending
            emit_norm_apply(lh_, yfull_, emit_norm_recip(yfull_))

    if PHASES < 3:
        pers_cm.__exit__(None, None, None)
        return

    # =============== Phase 3: 8-way AllToAll ===============
    # a2a_in [8 dest slots, 256 feat, 512 u] was packed inside the attention
    # phase (per-lh, as soon as each head pair was normalized): chunk uc is
    # duplicated to dest slots uc and 4+uc (one lands on the same-batch core
    # that needs it; the copy received from cross-batch cores is discarded
    # downstream by zero blocks in this core's wo tensor). Features: 64*lh+d.
    pers_cm.__exit__(None, None, None)

    if not NO_COLLECTIVE:
        nc.gpsimd.collective_compute(
            "AllToAll",
            mybir.AluOpType.bypass,
            replica_groups=[[0, 1, 2, 3, 4, 5, 6, 7]],
            ins=[a2a_in.opt()],
            outs=[a2a_out.opt()],
        )

    if PHASES < 4:
        return

    # =============== Phase 4: Wo on received y, residual, MLP ===============
    with (
        tc.tile_pool(name="resp" + sfx, bufs=1) as resp,
        tc.tile_pool(name="mlp" + sfx, bufs=1) as mlpp,
    ):
        res1 = [resp.tile([128, 512], BF16, tag="res1", bufs=8,
                          name=f"res1_{k_}{sfx}") for k_ in range(8)]
        xres = [resp.tile([128, 512], F32, tag="xres", bufs=8,
                          name=f"xres{k_}{sfx}") for k_ in range(8)]
        for m in range(8):
            nc.scalar.dma_start(xres[m][:], P["xres"][m])
        # y^T tiles: a2a_out[s] = 256 features of source core s for this
        # core's 512 u's. wo_sb[k] holds the matching Wo rows for same-batch
        # sources and zeros for cross-batch sources, so contracting over all
        # 16 tiles yields exactly this core's batch.
        yt = [resp.tile([128, 512], BF16, tag="yt", bufs=16,
                        name=f"yt{k_}{sfx}") for k_ in range(16)]
        for k in range(16):
            nc.scalar.dma_start(
                yt[k][:], a2a_out[k // 2, (k % 2) * 128:(k % 2) * 128 + 128, :]
            )
        with tc.tile_pool(name="wops" + sfx, bufs=2, space="PSUM") as wops:
            for m in range(8):
                ps = wops.tile([128, 512], F32, tag="wo", bufs=2)
                for k in range(16):
                    nc.tensor.matmul(
                        ps[:], wo_sb[k][:, m * 128:(m + 1) * 128], yt[k][:],
                        start=(k == 0), stop=(k == 15),
                    )
                nc.vector.scalar_tensor_tensor(
                    res1[m][:], ps[:], biases[:, 16 + m:17 + m], xres[m][:],
                    mybir.AluOpType.add, mybir.AluOpType.add,
                )

        if MLP_LEVEL < 2:
            return

        h1 = [mlpp.tile([128, 512], BF16, tag="h1", bufs=32, name=f"h1_{k_}{sfx}")
              for k_ in range(32)]
        h1ps_cm = tc.tile_pool(name="h1ps" + sfx, bufs=2, space="PSUM")
        mlpps = h1ps_cm.__enter__()
        for q in range(4):
            wf = [mlpp.tile([128, 1024], BF16, tag="wfc", bufs=16,
                            name=f"wf{q}_{k_}{sfx}") for k_ in range(8)]
            for k in range(8):
                nc.sync.dma_start(wf[k][:], P["wfc"][q, k])
            for mi in range(8):
                mt = q * 8 + mi
                ps = mlpps.tile([128, 512], F32, tag="h1ps", bufs=2)
                for k in range(8):
                    nc.tensor.matmul(
                        ps[:], wf[k][:, mi * 128:(mi + 1) * 128], res1[k][:],
                        start=(k == 0), stop=(k == 7),
                    )
                nc.scalar.activation(
                    h1[mt][:], ps[:], AF.Gelu_apprx_tanh,
                    bias=biases[:, 24 + mt:25 + mt],
                )
        h1ps_cm.__exit__(None, None, None)

        if MLP_LEVEL < 3:
            return

        projps_cm = tc.tile_pool(name="projps" + sfx, bufs=8, space="PSUM")
        projps = projps_cm.__enter__()
        pps = [projps.tile([128, 512], F32, tag="proj", bufs=8,
                           name=f"pps{k_}{sfx}") for k_ in range(8)]
        for k in range(32):
            wp = mlpp.tile([128, 1024], BF16, tag="wproj", bufs=3)
            nc.sync.dma_start(wp[:], P["wproj"][k])
            for m in range(8):
                nc.tensor.matmul(
                    pps[m][:], wp[:, m * 128:(m + 1) * 128], h1[k][:],
                    start=(k == 0), stop=(k == 31),
                )
        for m in range(8):
            ob = mlpp.tile([128, 512], F32, tag="osb", bufs=2)
            nc.vector.scalar_tensor_tensor(
                ob[:], pps[m][:], biases[:, 56 + m:57 + m], res1[m][:],
                mybir.AluOpType.add, mybir.AluOpType.add,
            )
            nc.sync.dma_start(out_p[m], ob[:])
        projps_cm.__exit__(None, None, None)


def _build(iters=1):
    nc = bacc.Bacc(None, target_bir_lowering=False, debug=True, num_devices=8)

    P = {}
    P["xt"] = nc.declare_dram_parameter("xt", [8, 128, 512], BF16, isOutput=False)
    P["xres"] = nc.declare_dram_parameter("xres", [8, 128, 512], F32, isOutput=False)
    P["wqk"] = nc.declare_dram_parameter("wqk", [2, 8, 128, 1024], BF16, isOutput=False)
    P["wv"] = nc.declare_dram_parameter("wv", [8, 128, 1024], BF16, isOutput=False)
    P["bv"] = nc.declare_dram_parameter("bv", [1, 1024], BF16, isOutput=False)
    P["wo"] = nc.declare_dram_parameter("wo", [16, 128, 1024], BF16, isOutput=False)
    P["wfc"] = nc.declare_dram_parameter("wfc", [4, 8, 128, 1024], BF16, isOutput=False)
    P["wproj"] = nc.declare_dram_parameter("wproj", [32, 128, 1024], BF16, isOutput=False)
    P["biases"] = nc.declare_dram_parameter("biases", [128, 64], F32, isOutput=False)
    P["masks"] = nc.declare_dram_parameter("masks", [5, 128, 512], BF16, isOutput=False)
    out_p = nc.declare_dram_parameter("out", [8, 128, 512], F32, isOutput=True)

    with tile.TileContext(nc) as tc:
        with (
            tc.tile_pool(name="const", bufs=1) as constp,
            tc.tile_pool(name="dram", bufs=1, space="DRAM") as dram,
        ):
            biases = constp.tile([128, 64], F32, tag="biases", bufs=1)
            nc.sync.dma_start(biases[:], P["biases"][:])
            masks = [constp.tile([128, 512], BF16, tag="masks", bufs=5,
                                 name=f"masks{k_}") for k_ in range(5)]
            for k in range(5):
                nc.sync.dma_start(masks[k][:], P["masks"][k])
            ones_f = constp.tile([1, 128], F32, tag="ones_f", bufs=1)
            nc.any.memset(ones_f[:], 1.0)
            ones_r = constp.tile([1, 128], F32R, tag="ones_r", bufs=1)
            nc.scalar.copy(ones_r[:], ones_f[:])
            ones_b = constp.tile([1, 128], BF16, tag="ones_b", bufs=1)
            nc.scalar.copy(ones_b[:], ones_f[:])
            bv = constp.tile([1, 1024], BF16, tag="bv", bufs=1)
            nc.sync.dma_start(bv[:], P["bv"][:])
            wo_sb = [constp.tile([128, 1024], BF16, tag="wo", bufs=16,
                                 name=f"wo{k_}") for k_ in range(16)]
            for p_ in range(16):
                nc.sync.dma_start(wo_sb[p_][:], P["wo"][p_])

            a2a_in = dram.tile([8, 256, 512], BF16, tag="a2a_in", bufs=1)
            a2a_out = dram.tile([8, 256, 512], BF16, tag="a2a_out", bufs=1)

            consts = (biases, masks, ones_r, ones_b, bv, wo_sb, a2a_in, a2a_out)
            for it in range(iters):
                _emit_body(nc, tc, P, out_p, consts, it)

    nc.finalize()
    return nc


_NC = None


def _get_nc():
    global _NC
    if _NC is None:
        _NC = _build()
    return _NC


def _prep_inputs(x, Wqkv, bqkv, Wo, bo, Wfc, bfc, Wproj, bproj):
    x = np.asarray(x, dtype=np.float32)
    Wqkv = np.asarray(Wqkv, dtype=np.float32)
    bqkv = np.asarray(bqkv, dtype=np.float32)
    Wo_ = np.asarray(Wo, dtype=np.float32)
    Wfc = np.asarray(Wfc, dtype=np.float32)
    bfc = np.asarray(bfc, dtype=np.float32)
    Wproj = np.asarray(Wproj, dtype=np.float32)

    wqk = Wqkv[:, :2048].reshape(8, 128, 2, 1024).transpose(2, 0, 1, 3)
    wqk = np.ascontiguousarray(wqk.astype(ml_dtypes.bfloat16))
    wv = np.ascontiguousarray(
        Wqkv[:, 2048:].reshape(8, 128, 1024).astype(ml_dtypes.bfloat16))
    bv = bqkv[2048:].reshape(1, 1024).astype(ml_dtypes.bfloat16)
    wfc = Wfc.reshape(8, 128, 4, 1024).transpose(2, 0, 1, 3)
    wfc = np.ascontiguousarray(wfc.astype(ml_dtypes.bfloat16))
    wproj = np.ascontiguousarray(
        Wproj.reshape(32, 128, 1024).astype(ml_dtypes.bfloat16))

    r_ = np.arange(128)
    strict = (r_[:, None] > r_[None, :]).astype(np.float32)
    incl = (r_[:, None] >= r_[None, :]).astype(np.float32)
    masks = np.zeros((5, 128, 512), np.float32)
    for k in range(5):
        for c in range(4):
            masks[k][:, c * 128:(c + 1) * 128] = (strict if c < k else incl).T
    masks = masks.astype(ml_dtypes.bfloat16)

    wo_tiles = np.ascontiguousarray(
        Wo_.reshape(8, 128, 1024).astype(ml_dtypes.bfloat16))
    # Per-batch wo tensor [16,128,1024]: tile k matches A2A source slot
    # s=k//2 (features 256*(s%4)+128*(k%2)); zero for cross-batch slots.
    wo16 = []
    for b in range(2):
        w = np.zeros((16, 128, 1024), dtype=ml_dtypes.bfloat16)
        for k in range(16):
            s = k // 2
            if s // 4 == b:
                w[k] = wo_tiles[2 * (s % 4) + (k % 2)]
        wo16.append(w)

    biases = np.zeros((128, 64), np.float32)
    biases[:, 0:16] = bqkv[:2048].reshape(16, 128).T
    biases[:, 16:24] = np.asarray(bo, dtype=np.float32).reshape(8, 128).T
    biases[:, 24:56] = bfc.reshape(32, 128).T
    biases[:, 56:64] = np.asarray(bproj, dtype=np.float32).reshape(8, 128).T

    in_maps = []
    for i in range(8):
        j, b = i % 4, i // 4
        xq = x[b, 512 * j:512 * (j + 1), :].T.astype(ml_dtypes.bfloat16)
        xr = np.ascontiguousarray(x[b, _u_rows(j), :].T)
        in_maps.append({
            "xt": np.ascontiguousarray(xq.reshape(8, 128, 512)),
            "xres": np.ascontiguousarray(xr.reshape(8, 128, 512)),
            "wqk": wqk, "wv": wv, "bv": bv,
            "wo": wo16[b], "wfc": wfc, "wproj": wproj,
            "biases": biases, "masks": masks,
        })
    return in_maps


def _assemble(results, dtype):
    out = np.empty((B, T, C), dtype=np.float32)
    for i in range(8):
        j, b = i % 4, i // 4
        o = results[i]["out"].reshape(1024, 512)
        out[b, _u_rows(j), :] = o.T
    return out.astype(dtype, copy=False)


def kernel(**inputs):
    nc = _get_nc()
    in_maps = _prep_inputs(**inputs)
    res = run_bass_kernel_spmd(nc, in_maps, core_ids=list(range(8)))
    return _assemble(res.results, np.asarray(inputs["x"]).dtype)


if __name__ == "__main__":
    _get_nc()
    print("build ok")

